# revision 39
# baseline (speedup 1.0000x reference)
"""Additive (Bahdanau-style) attention kernel for Trainium2, 8 NeuronCores.

reference computation (per batch b of 32, T=1024 timesteps, H=1024):
    mlp_hidden = selu([enc[b,t]; hid[b]] @ W1 + b1)     # (T, H)
    scores     = mlp_hidden @ W2 + b2                   # (T, 1)
    weights    = softmax(scores over t)
    out[b]     = sum_t weights[t] * enc[b,t]            # (H,)

Distribution: data-parallel over batch, 4 batches per core, no collectives.

Per-core algorithm (shard shapes):
  - enc @ W1 splits: enc @ W1[:H] + hid @ W1[H:]; the second term ("hid_part")
    is per-batch constant, computed once with hidT as the (4-wide) stationary.
  - b2 and selu's additive constant are dropped (softmax shift invariance);
    selu's lambda is folded into W2 on the host.
  - the big enc @ W1a matmul runs in fp8e4 DoubleRow mode (2 contraction
    chunks per instruction at 2 rows/cycle).  W1a is pre-scaled by 32 on the
    host so its values sit in fp8e4's normal range; the 32x then rides
    through the whole selu/score pipeline (relu is linear, exp gets a 2^-5
    activation scale, and the final softmax exp unscales the scores).
  - scores: w2-stationary 512-col streams over the selu tiles, deferred one
    j-chunk behind the mlp so the PE never waits on the elementwise chain.
  - selu elementwise work (GpSimd cannot touch PSUM and is ~15x slower than
    DVE on dense tiles, so it only runs the cast DMAs):
      ScalarE: e2' = exp(2^-5 * mp + hid + b1 + ln(32a))   (= 32a * e^y)
      ScalarE: r2' = relu(mp + 32*(hid + b1))              (= 32 * relu(y))
      VectorE: s2' = min(e2', 32a) + r2'                   (fused STT)
  - the eT transposes for the next t-group are emitted one k-chunk per
    j-slot between mlp groups, keeping PE / ScalarE / VectorE in ~1.6us
    lockstep instead of burst-stalling at t-group boundaries.
  - softmax: exp of the scores psum directly with a 2^-5 scale (scores are
    O(1); no max subtraction); 1/Z is folded into the output copy.
  - context = w @ E with E SBUF-resident bf16 (single HBM read of the
    encoder); each batch's epilogue is deferred behind the next batch's
    first mlp groups (split in two so the weight transposes never wait on
    the softmax exp) and the PE FIFO never stalls on it.
"""

import math

import ml_dtypes
import numpy as np

import concourse.tile as tile
from concourse.masks import make_identity
from concourse import bacc, mybir
from concourse.bass_utils import run_bass_kernel_spmd

F32 = mybir.dt.float32
BF16 = mybir.dt.bfloat16
FP8 = mybir.dt.float8e4
ALU = mybir.AluOpType
ACTF = mybir.ActivationFunctionType
DR = mybir.MatmulPerfMode.DoubleRow

N_CORES = 8
B = 32
T = 1024
H = 1024
BL = B // N_CORES          # batches per core = 4
KC = H // 128              # contraction chunks = 8
KP = KC // 2               # fp8 DoubleRow contraction pair-chunks = 4
JC = H // 128              # hidden-unit chunks = 8
TGS = 512                  # t-group size (one psum bank of f32)
NTG = T // TGS             # t-groups per batch = 2
TT = TGS // 128            # 128-row t-subtiles per t-group = 4

SELU_LAMBDA = 1.0507009873554805
SELU_ALPHA = 1.6732632423543772
SCALE = 32.0               # fp8 W1a pre-scale (power of two)
RSCALE = 1.0 / SCALE
SA = SCALE * SELU_ALPHA
USE_FP8 = True             # fp8e4 DoubleRow mlp vs plain bf16 (debug)
MMDT = FP8 if USE_FP8 else BF16


def build_kernel():
    nc = bacc.Bacc("TRN2", target_bir_lowering=False, debug=False,
                   num_devices=N_CORES)

    enc = nc.dram_tensor("enc", [BL, KC, 128, H], F32, kind="ExternalInput").ap()
    hidT = nc.dram_tensor("hidT", [KC, 128, BL], BF16, kind="ExternalInput").ap()
    w1a8 = nc.dram_tensor("w1a8", [KP, 128, 2, H], MMDT, kind="ExternalInput").ap()
    w1b = nc.dram_tensor("w1b", [KC, 128, H], BF16, kind="ExternalInput").ap()
    b1s = nc.dram_tensor("b1s", [128, JC], F32, kind="ExternalInput").ap()
    b1e = nc.dram_tensor("b1e", [128, JC], F32, kind="ExternalInput").ap()
    w2l = nc.dram_tensor("w2l", [128, JC], BF16, kind="ExternalInput").ap()
    out = nc.dram_tensor("out", [BL, H], F32, kind="ExternalOutput").ap()

    with tile.TileContext(nc) as tc:
        with (
            tc.tile_pool(name="consts", bufs=1) as consts,
            tc.tile_pool(name="encp", bufs=3) as encp,
            tc.tile_pool(name="etp", bufs=2) as etp,
            tc.tile_pool(name="selu", bufs=4) as selup,
            tc.tile_pool(name="score", bufs=2) as scorep,
            tc.tile_pool(name="outp", bufs=2) as outp,
            tc.tile_pool(name="psum", bufs=2, space="PSUM") as psum,
        ):
            # identity + PE warmup first: the warmup keeps the TensorE
            # activity monitor busy so the clock ungates before real work.
            identity = consts.tile([128, 128], BF16)
            make_identity(nc, identity)
            id4 = consts.tile([4, 4], F32)
            make_identity(nc, id4)
            one1 = consts.tile([1, 1], F32)
            nc.vector.memset(one1, 1.0)
            junk = consts.tile([128, 128], BF16)
            make_identity(nc, junk)
            warm_ps = psum.tile([128, 128], BF16, tag="mlp", bufs=3)
            for _ in range(16):
                nc.tensor.transpose(warm_ps, junk, junk)

            # --- replicated weights (HWDGE ring); hidT/w1b first so the
            # hid chain is never the head-of-line DMA wait.
            hidT_sb = consts.tile([128, KC, BL], BF16)
            nc.sync.dma_start(out=hidT_sb, in_=hidT.rearrange("k p b -> p k b"))
            b1s_sb = consts.tile([128, JC], F32)
            nc.sync.dma_start(out=b1s_sb, in_=b1s)
            b1e_sb = consts.tile([128, JC], F32)
            nc.sync.dma_start(out=b1e_sb, in_=b1e)
            w2l_sb = consts.tile([128, JC], BF16)
            nc.sync.dma_start(out=w2l_sb, in_=w2l)
            w1b_ts = []
            for k in range(KC):
                w1b_k = consts.tile([128, H], BF16, name=f"w1b_{k}")
                nc.sync.dma_start(out=w1b_k, in_=w1b[k])
                w1b_ts.append(w1b_k)
            w1a_ts = []
            for kp in range(KP):
                w1a_k = consts.tile([128, 2, H], MMDT, name=f"w1a_{kp}")
                nc.sync.dma_start(out=w1a_k, in_=w1a8[kp])
                w1a_ts.append(w1a_k)

            # encoder loads: gpsimd cast DMAs (only gpsimd DMAs can cast
            # f32 -> bf16).  gpsimd also runs the eT psum->fp8 copies, so
            # descriptor-gen for batch b+2 is emitted during batch b to
            # keep transfers ahead of the transposes.
            e_ts_all = [None] * BL

            def emit_loads(b):
                e_ts = []
                for tt in range(KC):
                    e_t = encp.tile([128, H], BF16, tag="e", bufs=4 * KC,
                                    name=f"e_{b}_{tt}")
                    nc.gpsimd.dma_start(out=e_t, in_=enc[b, tt])
                    e_ts.append(e_t)
                e_ts_all[b] = e_ts

            # per-(j,b) selu biases, filled by the hid chain below
            hb32 = consts.tile([128, JC, BL], F32)   # 32*(hid + b1)
            hbe = consts.tile([128, JC, BL], F32)    # hid + b1 + ln(32*alpha)

            def emit_hid():
                # hid_ps = 32 * hid_part, computed with the 4-wide hidT as
                # stationary so weight loads are 4 rows instead of 128.
                hid_sb = scorep.tile([4, H], F32, tag="hid", bufs=1)
                for half in range(2):
                    hp = psum.tile([4, TGS], F32, tag="ctx", bufs=1)
                    for k in range(KC):
                        nc.tensor.matmul(
                            hp,
                            lhsT=hidT_sb[:, k, :],
                            rhs=w1b_ts[k][:, half * TGS:(half + 1) * TGS],
                            start=(k == 0),
                            stop=(k == KC - 1),
                        )
                    nc.vector.tensor_copy(
                        out=hid_sb[:, half * TGS:(half + 1) * TGS], in_=hp)
                for j in range(JC):
                    tp4 = psum.tile([128, 4], F32, tag="ctx", bufs=1)
                    nc.tensor.transpose(
                        tp4, hid_sb[:, j * 128:(j + 1) * 128], id4)
                    nc.scalar.activation(
                        out=hb32[:, j, :], in_=tp4,
                        func=ACTF.Identity, bias=b1s_sb[:, j:j + 1], scale=1.0,
                    )
                    nc.scalar.activation(
                        out=hbe[:, j, :], in_=tp4,
                        func=ACTF.Identity, bias=b1e_sb[:, j:j + 1],
                        scale=RSCALE,
                    )

            # E^T pair tiles, keyed (b, tg); each holds [128h, kk, 512t]
            # for one kp.  Built one k-chunk at a time, interleaved between
            # mlp j-groups so the V cast copies never burst-stall the PE.
            eT_store = {}

            def emit_transpose_tile(b, tg, k):
                e_ts = e_ts_all[b]
                kp, kk = divmod(k, 2)
                if kk == 0:
                    eT_store.setdefault((b, tg), [None] * KP)[kp] = etp.tile(
                        [128, 2, TGS], MMDT, tag="eT", bufs=3 * KP,
                        name=f"eT_{b}_{tg}_{kp}")
                eT_k = eT_store[(b, tg)][kp]
                tp = psum.tile([128, TGS], BF16, tag="trans", bufs=2)
                for tt in range(TT):
                    t_idx = tg * TT + tt
                    nc.tensor.transpose(
                        tp[:, tt * 128:(tt + 1) * 128],
                        e_ts[t_idx][:, k * 128:(k + 1) * 128],
                        identity,
                    )
                nc.vector.tensor_copy(out=eT_k[:, kk, :], in_=tp)

            def emit_wcol(b, expw, w_col, lo, hi):
                # fp32 PE transposes of expw columns [lo, hi) -> w_col
                w_ps = psum.tile([128, hi - lo], F32, tag="sc", bufs=2,
                                 name=f"wps_{b}_{lo}")
                for c in range(lo, hi):
                    nc.tensor.transpose(
                        w_ps[:, c - lo:c - lo + 1],
                        expw[0:1, c * 128:(c + 1) * 128],
                        one1,
                    )
                nc.vector.tensor_copy(out=w_col[:, lo:hi], in_=w_ps)

            def emit_ctx(b, w_col, cp, lo, hi):
                # unnormalized context accumulation over t-chunks [lo, hi);
                # both h-halves live in one [33, 512] psum tile (partitions
                # 0 and 32) under a single accumulation group so the bank is
                # zeroed exactly once.
                e_ts = e_ts_all[b]
                for half in range(2):
                    for tch in range(lo, hi):
                        nc.tensor.matmul(
                            cp[32 * half:32 * half + 1, :],
                            lhsT=w_col[:, tch:tch + 1],
                            rhs=e_ts[tch][:, half * TGS:(half + 1) * TGS],
                            start=(tch == 0),
                            stop=(tch == KC - 1),
                            skip_group_check=True,
                        )

            def phase1(b, first=False, epi_hook=None):
                expw = scorep.tile([1, T], F32, tag="expw", bufs=2,
                                   name=f"expw_{b}")
                w_col = scorep.tile([128, KC], BF16, tag="wcol", bufs=2,
                                    name=f"wcol_{b}")
                cp = psum.tile([33, TGS], F32, tag="ctx", bufs=1,
                               name=f"cp_{b}")
                rsums = []
                pend = [None]   # deferred score dot (j, tg, s2, sc_ps)
                ctxa = [None]   # tg0 context half, pending until exp lands

                def emit_dot():
                    # w2-stationary 512-col stream; when a t-group's last
                    # chunk lands, its softmax exp row follows on ScalarE.
                    j, tg, s2, sc_ps = pend[0]
                    pend[0] = None
                    nc.tensor.matmul(
                        sc_ps,
                        lhsT=w2l_sb[:, j:j + 1],
                        rhs=s2,
                        start=(j == 0),
                        stop=(j == JC - 1),
                    )
                    if j == JC - 1:
                        rs = scorep.tile([1, 1], F32, tag=f"rs{tg}", bufs=2)
                        nc.scalar.activation(
                            out=expw[:, tg * TGS:(tg + 1) * TGS], in_=sc_ps,
                            func=ACTF.Exp, scale=RSCALE, accum_out=rs)
                        rsums.append(rs)
                        if tg == 0:
                            ctxa[0] = 2  # emit tg0's context 2 slots later

                if first:
                    # batch 0: the first t-group's transposes must precede
                    # its matmul groups.
                    for k in range(KC):
                        emit_transpose_tile(b, 0, k)

                for tg in range(NTG):
                    eT_ps = eT_store[(b, tg)]
                    sc_ps = psum.tile([1, TGS], F32, tag="sc", bufs=2)
                    for j in range(JC):
                        mp = psum.tile([128, TGS], F32, tag="mlp", bufs=3)
                        for kp in range(KP):
                            nc.tensor.matmul(
                                mp,
                                lhsT=w1a_ts[kp][:, :, j * 128:(j + 1) * 128],
                                rhs=eT_ps[kp],
                                start=(kp == 0),
                                stop=(kp == KP - 1),
                                perf_mode=DR if USE_FP8 else None,
                            )
                        if tg == 0 and j == 0:
                            # previous batch's deferred scores + softmax ride
                            # behind this batch's first group (its context
                            # matmuls two groups later); batch 0 instead puts
                            # the hid chain here -- the hbe/hb32 writes must
                            # be emitted before any selu reads them.
                            if epi_hook is not None:
                                epi_hook(0)
                            elif first:
                                emit_hid()
                        if tg == 0 and j == 2 and epi_hook is not None:
                            epi_hook(1)
                        # build the next t-group's (or next batch's) eT
                        # tiles one k-chunk per j-slot, so the PE transposes
                        # and the V cast copies stay in lockstep with the
                        # mlp instead of burst-stalling at t-group bounds.
                        # k-chunk j+1 (plus both k=0,1 in the first slot):
                        # the cast copies get a one-slot head start so the
                        # next t-group's first mlp never waits on the last
                        # cast.
                        ks = [0, 1] if j == 0 else ([j + 1] if j < JC - 1 else [])
                        for k_ in ks:
                            if tg == 0:
                                emit_transpose_tile(b, 1, k_)
                            elif b + 1 < BL:
                                emit_transpose_tile(b + 1, 0, k_)
                        if pend[0] is not None:
                            emit_dot()
                        if ctxa[0] is not None:
                            # tg0's softmax half is ready: transpose its
                            # weight columns and run its context t-chunks
                            # here, hidden under tg1's mlp groups (and off
                            # the deferred epilogue's critical path).
                            ctxa[0] -= 1
                            if ctxa[0] == 0:
                                ctxa[0] = None
                                emit_wcol(b, expw, w_col, 0, TT)
                                emit_ctx(b, w_col, cp, 0, TT)
                        e2 = selup.tile([128, TGS], BF16, tag="e2", bufs=4)
                        nc.scalar.activation(out=e2, in_=mp, func=ACTF.Exp,
                                             bias=hbe[:, j, b:b + 1],
                                             scale=RSCALE)
                        r2 = selup.tile([128, TGS], BF16, tag="r2", bufs=4)
                        nc.scalar.activation(out=r2, in_=mp, func=ACTF.Relu,
                                             bias=hb32[:, j, b:b + 1],
                                             scale=1.0)
                        # fused clamp+combine in one DVE pass:
                        # s2 = min(e2, 32a) + r2
                        s2 = selup.tile([128, TGS], BF16, tag="s2", bufs=6)
                        nc.vector.scalar_tensor_tensor(
                            out=s2, in0=e2, scalar=SA, in1=r2,
                            op0=ALU.min, op1=ALU.add,
                        )
                        pend[0] = (j, tg, s2, sc_ps)
                return expw, w_col, cp, rsums, emit_dot, []

            def epilogue(b, state, part):
                # Deferred one batch and split in two: part 0 (final score
                # dot + softmax exp + normalizer) after the next batch's
                # first mlp group, part 1 (tg1 weight transposes + context
                # tail) one group later so the transposes never wait on the
                # exp.  The tg0 context half already ran inside batch b.
                expw, w_col, cp, rsums, emit_dot, stash = state
                if part == 0:
                    emit_dot()
                    rsum = scorep.tile([1, 1], F32, tag="rsum")
                    nc.vector.tensor_add(out=rsum, in0=rsums[0],
                                         in1=rsums[1])
                    rinv = scorep.tile([1, 1], F32, tag="rinv")
                    nc.vector.reciprocal(rinv, rsum)
                    stash.append(rinv)
                    return
                rinv = stash[0]

                emit_wcol(b, expw, w_col, TT, KC)
                emit_ctx(b, w_col, cp, TT, KC)
                # normalization by 1/Z is folded into the output copy
                ob = outp.tile([1, H], F32, tag="ob")
                for half in range(2):
                    nc.scalar.activation(
                        out=ob[:, half * TGS:(half + 1) * TGS],
                        in_=cp[32 * half:32 * half + 1, :],
                        func=ACTF.Copy, scale=rinv)
                nc.sync.dma_start(out=out[b:b + 1, :], in_=ob)

            emit_loads(0)
            emit_loads(1)
            prev = None
            for b in range(BL):
                if b + 2 < BL:
                    emit_loads(b + 2)
                hook = None
                if prev is not None:
                    hook = (lambda part, pb=b - 1, ps=prev: epilogue(
                        pb, ps, part))
                prev = phase1(b, first=(b == 0), epi_hook=hook)
            epilogue(BL - 1, prev, 0)
            epilogue(BL - 1, prev, 1)

    nc.compile()
    return nc


_NC_CACHE = None


def _get_nc():
    global _NC_CACHE
    if _NC_CACHE is None:
        _NC_CACHE = build_kernel()
    return _NC_CACHE


def make_in_maps(encoder_outputs, hidden_state, W1, b1, W2):
    enc = np.ascontiguousarray(np.asarray(encoder_outputs, np.float32))
    hid = np.ascontiguousarray(np.asarray(hidden_state, np.float32))
    W1 = np.asarray(W1, np.float32)
    b1 = np.asarray(b1, np.float32)
    W2 = np.asarray(W2, np.float32)

    bf16 = ml_dtypes.bfloat16
    fp8 = ml_dtypes.float8_e4m3
    mmdt = fp8 if USE_FP8 else bf16
    # [kp, p, kk, j] = 32 * W1a[(2kp+kk)*128 + p, j]
    w1a8 = np.ascontiguousarray(
        (W1[:H] * SCALE).reshape(KP, 2, 128, H).transpose(0, 2, 1, 3)
    ).astype(mmdt)
    w1b = np.ascontiguousarray(W1[H:] * SCALE).reshape(KC, 128, H).astype(bf16)
    b1c = b1.reshape(KC, 128).T                                   # (128, JC)
    b1s = np.ascontiguousarray(SCALE * b1c).astype(np.float32)
    b1e = np.ascontiguousarray(
        b1c + math.log(SCALE * SELU_ALPHA)).astype(np.float32)
    w2l = np.ascontiguousarray(
        (W2[:, 0] * SELU_LAMBDA).reshape(JC, 128).T).astype(bf16)

    in_maps = []
    for c in range(N_CORES):
        sl = slice(BL * c, BL * (c + 1))
        in_maps.append({
            "enc": np.ascontiguousarray(enc[sl]).reshape(BL, KC, 128, H),
            "hidT": np.ascontiguousarray(hid[0, sl].T).reshape(KC, 128, BL)
                      .astype(bf16),
            "w1a8": w1a8,
            "w1b": w1b,
            "b1s": b1s,
            "b1e": b1e,
            "w2l": w2l,
        })
    return in_maps


def kernel(encoder_outputs, hidden_state, W1, b1, W2, b2):
    # b2 shifts every score equally; softmax is shift-invariant, so it is
    # deliberately unused.
    in_maps = make_in_maps(encoder_outputs, hidden_state, W1, b1, W2)
    nc = _get_nc()
    res = run_bass_kernel_spmd(nc, in_maps, core_ids=list(range(N_CORES)))
    out = np.empty((1, B, H), np.float32)
    for c in range(N_CORES):
        out[0, BL * c:BL * (c + 1)] = res.results[c]["out"]
    return out


# revision 40
# speedup vs baseline: 1.0229x; 1.0229x over previous
"""Additive (Bahdanau-style) attention kernel for Trainium2, 8 NeuronCores.

reference computation (per batch b of 32, T=1024 timesteps, H=1024):
    mlp_hidden = selu([enc[b,t]; hid[b]] @ W1 + b1)     # (T, H)
    scores     = mlp_hidden @ W2 + b2                   # (T, 1)
    weights    = softmax(scores over t)
    out[b]     = sum_t weights[t] * enc[b,t]            # (H,)

Distribution: data-parallel over batch, 4 batches per core, no collectives.

Per-core algorithm (shard shapes):
  - enc @ W1 splits: enc @ W1[:H] + hid @ W1[H:]; the second term ("hid_part")
    is per-batch constant, computed once with hidT as the (4-wide) stationary.
  - b2 and selu's additive constant are dropped (softmax shift invariance);
    selu's lambda is folded into W2 on the host.
  - the big enc @ W1a matmul runs in fp8e4 DoubleRow mode (2 contraction
    chunks per instruction at 2 rows/cycle).  W1a is pre-scaled by 32 on the
    host so its values sit in fp8e4's normal range; the 32x then rides
    through the whole selu/score pipeline (relu is linear, exp gets a 2^-5
    activation scale, and the final softmax exp unscales the scores).
  - scores: w2-stationary 512-col streams over the selu tiles, deferred one
    j-chunk behind the mlp so the PE never waits on the elementwise chain.
  - selu elementwise work (GpSimd cannot touch PSUM and is ~15x slower than
    DVE on dense tiles, so it only runs the cast DMAs):
      ScalarE: e2' = exp(2^-5 * mp + hid + b1 + ln(32a))   (= 32a * e^y)
      ScalarE: r2' = relu(mp + 32*(hid + b1))              (= 32 * relu(y))
      VectorE: s2' = min(e2', 32a) + r2'                   (fused STT)
  - the eT transposes for the next t-group are emitted one k-chunk per
    j-slot between mlp groups, keeping PE / ScalarE / VectorE in ~1.6us
    lockstep instead of burst-stalling at t-group boundaries.
  - softmax: exp of the scores psum directly with a 2^-5 scale (scores are
    O(1); no max subtraction); 1/Z is folded into the output copy.
  - context = w @ E with E SBUF-resident bf16 (single HBM read of the
    encoder); each batch's epilogue is deferred behind the next batch's
    first mlp groups (split in two so the weight transposes never wait on
    the softmax exp) and the PE FIFO never stalls on it.
"""

import math

import ml_dtypes
import numpy as np

import concourse.tile as tile
from concourse.masks import make_identity
from concourse import bacc, mybir
from concourse.bass_utils import run_bass_kernel_spmd

F32 = mybir.dt.float32
BF16 = mybir.dt.bfloat16
FP8 = mybir.dt.float8e4
ALU = mybir.AluOpType
ACTF = mybir.ActivationFunctionType
DR = mybir.MatmulPerfMode.DoubleRow

N_CORES = 8
B = 32
T = 1024
H = 1024
BL = B // N_CORES          # batches per core = 4
KC = H // 128              # contraction chunks = 8
KP = KC // 2               # fp8 DoubleRow contraction pair-chunks = 4
JC = H // 128              # hidden-unit chunks = 8
TGS = 512                  # t-group size (one psum bank of f32)
NTG = T // TGS             # t-groups per batch = 2
TT = TGS // 128            # 128-row t-subtiles per t-group = 4

SELU_LAMBDA = 1.0507009873554805
SELU_ALPHA = 1.6732632423543772
SCALE = 32.0               # fp8 W1a pre-scale (power of two)
RSCALE = 1.0 / SCALE
SA = SCALE * SELU_ALPHA
USE_FP8 = True             # fp8e4 DoubleRow mlp vs plain bf16 (debug)
MMDT = FP8 if USE_FP8 else BF16


def build_kernel():
    nc = bacc.Bacc("TRN2", target_bir_lowering=False, debug=False,
                   num_devices=N_CORES)

    enc = nc.dram_tensor("enc", [BL, KC, 128, H], F32, kind="ExternalInput").ap()
    hidT = nc.dram_tensor("hidT", [KC, 128, BL], BF16, kind="ExternalInput").ap()
    w1a8 = nc.dram_tensor("w1a8", [KP, 128, 2, H], MMDT, kind="ExternalInput").ap()
    w1b = nc.dram_tensor("w1b", [KC, 128, H], BF16, kind="ExternalInput").ap()
    b1s = nc.dram_tensor("b1s", [128, JC], F32, kind="ExternalInput").ap()
    b1e = nc.dram_tensor("b1e", [128, JC], F32, kind="ExternalInput").ap()
    w2l = nc.dram_tensor("w2l", [128, JC], BF16, kind="ExternalInput").ap()
    out = nc.dram_tensor("out", [BL, H], F32, kind="ExternalOutput").ap()

    with tile.TileContext(nc) as tc:
        with (
            tc.tile_pool(name="consts", bufs=1) as consts,
            tc.tile_pool(name="encp", bufs=3) as encp,
            tc.tile_pool(name="etp", bufs=2) as etp,
            tc.tile_pool(name="selu", bufs=4) as selup,
            tc.tile_pool(name="score", bufs=2) as scorep,
            tc.tile_pool(name="outp", bufs=2) as outp,
            tc.tile_pool(name="psum", bufs=2, space="PSUM") as psum,
        ):
            # identity + PE warmup first: the warmup keeps the TensorE
            # activity monitor busy so the clock ungates before real work.
            identity = consts.tile([128, 128], BF16)
            make_identity(nc, identity)
            id4 = consts.tile([4, 4], F32)
            make_identity(nc, id4)
            one1 = consts.tile([1, 1], F32)
            nc.vector.memset(one1, 1.0)
            junk = consts.tile([128, 128], BF16)
            make_identity(nc, junk)
            warm_ps = psum.tile([128, 128], BF16, tag="mlp", bufs=3)
            for _ in range(16):
                nc.tensor.transpose(warm_ps, junk, junk)

            # --- replicated weights (HWDGE ring); hidT/w1b first so the
            # hid chain is never the head-of-line DMA wait.
            hidT_sb = consts.tile([128, KC, BL], BF16)
            nc.sync.dma_start(out=hidT_sb, in_=hidT.rearrange("k p b -> p k b"))
            b1s_sb = consts.tile([128, JC], F32)
            nc.sync.dma_start(out=b1s_sb, in_=b1s)
            b1e_sb = consts.tile([128, JC], F32)
            nc.sync.dma_start(out=b1e_sb, in_=b1e)
            w2l_sb = consts.tile([128, JC], BF16)
            nc.sync.dma_start(out=w2l_sb, in_=w2l)
            w1b_ts = []
            for k in range(KC):
                w1b_k = consts.tile([128, H], BF16, name=f"w1b_{k}")
                nc.sync.dma_start(out=w1b_k, in_=w1b[k])
                w1b_ts.append(w1b_k)
            w1a_ts = []
            for kp in range(KP):
                w1a_k = consts.tile([128, 2, H], MMDT, name=f"w1a_{kp}")
                nc.sync.dma_start(out=w1a_k, in_=w1a8[kp])
                w1a_ts.append(w1a_k)

            # encoder loads: gpsimd cast DMAs (only gpsimd DMAs can cast
            # f32 -> bf16).  gpsimd also runs the eT psum->fp8 copies, so
            # descriptor-gen for batch b+2 is emitted during batch b to
            # keep transfers ahead of the transposes.
            e_ts_all = [None] * BL

            def emit_loads(b):
                e_ts = []
                for tt in range(KC):
                    e_t = encp.tile([128, H], BF16, tag="e", bufs=4 * KC,
                                    name=f"e_{b}_{tt}")
                    nc.gpsimd.dma_start(out=e_t, in_=enc[b, tt])
                    e_ts.append(e_t)
                e_ts_all[b] = e_ts

            # per-(j,b) selu biases, filled by the hid chain below
            hb32 = consts.tile([128, JC, BL], F32)   # 32*(hid + b1)
            hbe = consts.tile([128, JC, BL], F32)    # hid + b1 + ln(32*alpha)

            def emit_hid():
                # hid_ps = 32 * hid_part, computed with the 4-wide hidT as
                # stationary so weight loads are 4 rows instead of 128.
                hid_sb = scorep.tile([4, H], F32, tag="hid", bufs=1)
                for half in range(2):
                    hp = psum.tile([4, TGS], F32, tag="ctx", bufs=1)
                    for k in range(KC):
                        nc.tensor.matmul(
                            hp,
                            lhsT=hidT_sb[:, k, :],
                            rhs=w1b_ts[k][:, half * TGS:(half + 1) * TGS],
                            start=(k == 0),
                            stop=(k == KC - 1),
                        )
                    nc.vector.tensor_copy(
                        out=hid_sb[:, half * TGS:(half + 1) * TGS], in_=hp)
                for j in range(JC):
                    tp4 = psum.tile([128, 4], F32, tag="ctx", bufs=1)
                    nc.tensor.transpose(
                        tp4, hid_sb[:, j * 128:(j + 1) * 128], id4)
                    nc.scalar.activation(
                        out=hb32[:, j, :], in_=tp4,
                        func=ACTF.Identity, bias=b1s_sb[:, j:j + 1], scale=1.0,
                    )
                    nc.scalar.activation(
                        out=hbe[:, j, :], in_=tp4,
                        func=ACTF.Identity, bias=b1e_sb[:, j:j + 1],
                        scale=RSCALE,
                    )

            # E^T pair tiles, keyed (b, tg); each holds [128h, kk, 512t]
            # for one kp.  Built one k-chunk at a time, interleaved between
            # mlp j-groups so the V cast copies never burst-stall the PE.
            eT_store = {}

            def emit_transpose_tile(b, tg, k):
                e_ts = e_ts_all[b]
                kp, kk = divmod(k, 2)
                if kk == 0:
                    eT_store.setdefault((b, tg), [None] * KP)[kp] = etp.tile(
                        [128, 2, TGS], MMDT, tag="eT", bufs=3 * KP,
                        name=f"eT_{b}_{tg}_{kp}")
                eT_k = eT_store[(b, tg)][kp]
                tp = psum.tile([128, TGS], BF16, tag="trans", bufs=2)
                for tt in range(TT):
                    t_idx = tg * TT + tt
                    nc.tensor.transpose(
                        tp[:, tt * 128:(tt + 1) * 128],
                        e_ts[t_idx][:, k * 128:(k + 1) * 128],
                        identity,
                    )
                nc.vector.tensor_copy(out=eT_k[:, kk, :], in_=tp)

            def emit_wcol(b, expw, w_col, lo, hi):
                # fp32 PE transposes of expw columns [lo, hi) -> w_col
                w_ps = psum.tile([128, hi - lo], F32, tag="sc", bufs=2,
                                 name=f"wps_{b}_{lo}")
                for c in range(lo, hi):
                    nc.tensor.transpose(
                        w_ps[:, c - lo:c - lo + 1],
                        expw[0:1, c * 128:(c + 1) * 128],
                        one1,
                    )
                nc.vector.tensor_copy(out=w_col[:, lo:hi], in_=w_ps)

            def emit_ctx(b, w_col, cp, lo, hi):
                # unnormalized context accumulation over t-chunks [lo, hi);
                # both h-halves live in one [33, 512] psum tile (partitions
                # 0 and 32) under a single accumulation group so the bank is
                # zeroed exactly once.
                e_ts = e_ts_all[b]
                for half in range(2):
                    for tch in range(lo, hi):
                        nc.tensor.matmul(
                            cp[32 * half:32 * half + 1, :],
                            lhsT=w_col[:, tch:tch + 1],
                            rhs=e_ts[tch][:, half * TGS:(half + 1) * TGS],
                            start=(tch == 0),
                            stop=(tch == KC - 1),
                            skip_group_check=True,
                        )

            def phase1(b, first=False, epi_hook=None):
                expw = scorep.tile([1, T], F32, tag="expw", bufs=2,
                                   name=f"expw_{b}")
                w_col = scorep.tile([128, KC], BF16, tag="wcol", bufs=2,
                                    name=f"wcol_{b}")
                cp = psum.tile([33, TGS], F32, tag="ctx", bufs=1,
                               name=f"cp_{b}")
                rsums = []
                pend = [None]   # deferred score dot (j, tg, s2, sc_ps)
                ctxa = [None]   # tg0 context half, pending until exp lands

                def emit_dot():
                    # w2-stationary 512-col stream; when a t-group's last
                    # chunk lands, its softmax exp row follows on ScalarE.
                    j, tg, s2, sc_ps = pend[0]
                    pend[0] = None
                    nc.tensor.matmul(
                        sc_ps,
                        lhsT=w2l_sb[:, j:j + 1],
                        rhs=s2,
                        start=(j == 0),
                        stop=(j == JC - 1),
                    )
                    if j == JC - 1:
                        rs = scorep.tile([1, 1], F32, tag=f"rs{tg}", bufs=2)
                        nc.scalar.activation(
                            out=expw[:, tg * TGS:(tg + 1) * TGS], in_=sc_ps,
                            func=ACTF.Exp, scale=RSCALE, accum_out=rs)
                        rsums.append(rs)
                        if tg == 0:
                            ctxa[0] = 2  # emit tg0's context 2 slots later

                if first:
                    # batch 0: the first t-group's transposes must precede
                    # its matmul groups.
                    for k in range(KC):
                        emit_transpose_tile(b, 0, k)

                for tg in range(NTG):
                    eT_ps = eT_store[(b, tg)]
                    sc_ps = psum.tile([1, TGS], F32, tag="sc", bufs=2)
                    for j in range(JC):
                        mp = psum.tile([128, TGS], F32, tag="mlp", bufs=3)
                        for kp in range(KP):
                            nc.tensor.matmul(
                                mp,
                                lhsT=w1a_ts[kp][:, :, j * 128:(j + 1) * 128],
                                rhs=eT_ps[kp],
                                start=(kp == 0),
                                stop=(kp == KP - 1),
                                perf_mode=DR if USE_FP8 else None,
                            )
                        if tg == 0 and j == 0:
                            # previous batch's deferred scores + softmax ride
                            # behind this batch's first group (its context
                            # matmuls two groups later); batch 0 instead puts
                            # the hid chain here -- the hbe/hb32 writes must
                            # be emitted before any selu reads them.
                            if epi_hook is not None:
                                epi_hook(0)
                            elif first:
                                emit_hid()
                        if tg == 0 and j == 2 and epi_hook is not None:
                            epi_hook(1)
                        # build the next t-group's (or next batch's) eT
                        # tiles one k-chunk per j-slot, so the PE transposes
                        # and the V cast copies stay in lockstep with the
                        # mlp instead of burst-stalling at t-group bounds.
                        if tg == 0:
                            emit_transpose_tile(b, 1, j)
                        elif b + 1 < BL:
                            emit_transpose_tile(b + 1, 0, j)
                        if pend[0] is not None:
                            emit_dot()
                        if ctxa[0] is not None:
                            # tg0's softmax half is ready: transpose its
                            # weight columns and run its context t-chunks
                            # here, hidden under tg1's mlp groups (and off
                            # the deferred epilogue's critical path).
                            ctxa[0] -= 1
                            if ctxa[0] == 0:
                                ctxa[0] = None
                                emit_wcol(b, expw, w_col, 0, TT)
                                emit_ctx(b, w_col, cp, 0, TT)
                        e2 = selup.tile([128, TGS], BF16, tag="e2", bufs=4)
                        nc.scalar.activation(out=e2, in_=mp, func=ACTF.Exp,
                                             bias=hbe[:, j, b:b + 1],
                                             scale=RSCALE)
                        r2 = selup.tile([128, TGS], BF16, tag="r2", bufs=4)
                        nc.scalar.activation(out=r2, in_=mp, func=ACTF.Relu,
                                             bias=hb32[:, j, b:b + 1],
                                             scale=1.0)
                        # fused clamp+combine in one DVE pass:
                        # s2 = min(e2, 32a) + r2
                        s2 = selup.tile([128, TGS], BF16, tag="s2", bufs=6)
                        nc.vector.scalar_tensor_tensor(
                            out=s2, in0=e2, scalar=SA, in1=r2,
                            op0=ALU.min, op1=ALU.add,
                        )
                        pend[0] = (j, tg, s2, sc_ps)
                return expw, w_col, cp, rsums, emit_dot, []

            def epilogue(b, state, part):
                # Deferred one batch and split in two: part 0 (final score
                # dot + softmax exp + normalizer) after the next batch's
                # first mlp group, part 1 (tg1 weight transposes + context
                # tail) one group later so the transposes never wait on the
                # exp.  The tg0 context half already ran inside batch b.
                expw, w_col, cp, rsums, emit_dot, stash = state
                if part == 0:
                    emit_dot()
                    rsum = scorep.tile([1, 1], F32, tag="rsum")
                    nc.vector.tensor_add(out=rsum, in0=rsums[0],
                                         in1=rsums[1])
                    rinv = scorep.tile([1, 1], F32, tag="rinv")
                    nc.vector.reciprocal(rinv, rsum)
                    stash.append(rinv)
                    return
                rinv = stash[0]

                emit_wcol(b, expw, w_col, TT, KC)
                emit_ctx(b, w_col, cp, TT, KC)
                # normalization by 1/Z is folded into the output copy
                ob = outp.tile([1, H], F32, tag="ob")
                for half in range(2):
                    nc.scalar.activation(
                        out=ob[:, half * TGS:(half + 1) * TGS],
                        in_=cp[32 * half:32 * half + 1, :],
                        func=ACTF.Copy, scale=rinv)
                nc.sync.dma_start(out=out[b:b + 1, :], in_=ob)

            emit_loads(0)
            emit_loads(1)
            prev = None
            for b in range(BL):
                if b + 2 < BL:
                    emit_loads(b + 2)
                hook = None
                if prev is not None:
                    hook = (lambda part, pb=b - 1, ps=prev: epilogue(
                        pb, ps, part))
                prev = phase1(b, first=(b == 0), epi_hook=hook)
            epilogue(BL - 1, prev, 0)
            epilogue(BL - 1, prev, 1)

    nc.compile()
    return nc


_NC_CACHE = None


def _get_nc():
    global _NC_CACHE
    if _NC_CACHE is None:
        _NC_CACHE = build_kernel()
    return _NC_CACHE


def make_in_maps(encoder_outputs, hidden_state, W1, b1, W2):
    enc = np.ascontiguousarray(np.asarray(encoder_outputs, np.float32))
    hid = np.ascontiguousarray(np.asarray(hidden_state, np.float32))
    W1 = np.asarray(W1, np.float32)
    b1 = np.asarray(b1, np.float32)
    W2 = np.asarray(W2, np.float32)

    bf16 = ml_dtypes.bfloat16
    fp8 = ml_dtypes.float8_e4m3
    mmdt = fp8 if USE_FP8 else bf16
    # [kp, p, kk, j] = 32 * W1a[(2kp+kk)*128 + p, j]
    w1a8 = np.ascontiguousarray(
        (W1[:H] * SCALE).reshape(KP, 2, 128, H).transpose(0, 2, 1, 3)
    ).astype(mmdt)
    w1b = np.ascontiguousarray(W1[H:] * SCALE).reshape(KC, 128, H).astype(bf16)
    b1c = b1.reshape(KC, 128).T                                   # (128, JC)
    b1s = np.ascontiguousarray(SCALE * b1c).astype(np.float32)
    b1e = np.ascontiguousarray(
        b1c + math.log(SCALE * SELU_ALPHA)).astype(np.float32)
    w2l = np.ascontiguousarray(
        (W2[:, 0] * SELU_LAMBDA).reshape(JC, 128).T).astype(bf16)

    in_maps = []
    for c in range(N_CORES):
        sl = slice(BL * c, BL * (c + 1))
        in_maps.append({
            "enc": np.ascontiguousarray(enc[sl]).reshape(BL, KC, 128, H),
            "hidT": np.ascontiguousarray(hid[0, sl].T).reshape(KC, 128, BL)
                      .astype(bf16),
            "w1a8": w1a8,
            "w1b": w1b,
            "b1s": b1s,
            "b1e": b1e,
            "w2l": w2l,
        })
    return in_maps


def kernel(encoder_outputs, hidden_state, W1, b1, W2, b2):
    # b2 shifts every score equally; softmax is shift-invariant, so it is
    # deliberately unused.
    in_maps = make_in_maps(encoder_outputs, hidden_state, W1, b1, W2)
    nc = _get_nc()
    res = run_bass_kernel_spmd(nc, in_maps, core_ids=list(range(N_CORES)))
    out = np.empty((1, B, H), np.float32)
    for c in range(N_CORES):
        out[0, BL * c:BL * (c + 1)] = res.results[c]["out"]
    return out
